# revision 6
# baseline (speedup 1.0000x reference)
"""Trainium2 Bass kernel for nn_Min_interval (subset min-interval selection).

Problem: for each batch row, for every subset S of the 16 input columns with
|S| <= 3, output the (xl, xu) interval of the column in S minimizing the
score s = 0.5*xl + 0.5*xu (ties via b = 0.2*xl + 0.8*xu, then DP fold
order).  Output columns are sorted by subset bitmask -> [B, 696] x 2.

Packed-key min formulation (v2).  The baseline kernel materialized every
select as is_gt mask + plain copy + copy_predicated, which made the Vector
engine the bottleneck (~2650 DVE element-cycles/row, ~190 us vs the ~127 us
per-core HBM write roofline).  v2 packs key and payload into one fp32 so the
whole DP collapses to tensor_tensor MIN ops plus one fused custom-DVE op:

  *  key: Q = round(s * 2^14), s = xl+xu (compares identically to the
     0.5-weighted score).  Q in [0, 2^15], an exact fp32 integer.
  *  payload: v8 = clamp(round(v * 256), 0, 255) for v in {xl, xu}.
  *  packed: P = Q + v8/256 - exactly representable (<= 24 mantissa bits),
     so min(P) orders by (Q, v8) lexicographically and argmin Q = argmin s
     whenever all 16 row buckets are distinct.
  *  Rows where any two columns share a Q bucket (~0.5% of rows) are
     recomputed exactly on host; on all other rows the device winner equals
     the reference winner and the only error is payload quantization
     (<= 2^-9 absolute, vs the 2e-2 harness gate).
  *  Unpack is fused into the final min via a registered custom DVE op
     (single 1x pass, 8-slice ALU chain):
        m = min(in0, in1); out = m - (((m - 0.498046875) + 1.5*2^23) - 1.5*2^23)
     i.e. subtract away round-to-nearest-integer(m - (0.5 - 2^-9)) = Q of the
     winner, leaving v8/256 in [0,1).  The 0.5 - 2^-9 offset makes the
     round a truncation with no representable tie.
  *  Per chunk: pack (5 small DVE ops), 16 pair-stage TT mins building the
     136-col |S|<=2 staging array M2 (sentinel-prefixed, bitmask order), and
     16 fused min+unpack ops emitting the 696 output blocks, each a uniform
     min of a contiguous M2 prefix against one broadcast single.
  *  Outputs stream to HBM in two column waves per chunk on the SP and
     Activation HWDGE queues so stores overlap compute.

Sharding: batch 65536 rows -> 8 cores x 8192 rows (data parallel, no comms).
DVE work drops to ~30k cycles/chunk (~123 us/core total), at the HBM write
roofline for the 45.6 MB/core output.
"""

import os
import sys
import numpy as np

for _p in ("/opt/trn_rl_repo",):
    if _p not in sys.path and os.path.isdir(_p):
        sys.path.insert(0, _p)

# ----------------------------------------------------------------------------
# Problem constants (hardcoded per contest rules)
# ----------------------------------------------------------------------------
N = 16                 # input feature columns
ADD = 3                # max subset order
ALPHA = 0.5
BETA = 0.8
BATCH = 65536
N_CORES = 8
ROWS_PER_CORE = BATCH // N_CORES        # 8192
P = 128                                 # SBUF partitions
OUT_COLS = 696                          # subsets with 1<=|S|<=3 of 16
NB_DEFAULT = 16                         # rowblocks per chunk

# triangular prefix counts: c2[t] = #subsets |S|<=2 with bitmask < 2^t
C2 = [t * (t + 1) // 2 for t in range(N + 1)]
# output block offsets: block t (top bit t) starts at BOFF[t], width 1+c2[t]
BOFF = [0] * (N + 1)
for _t in range(N):
    BOFF[_t + 1] = BOFF[_t] + 1 + C2[_t]
assert BOFF[N] == OUT_COLS

M2_COLS = C2[N]  # 136 = number of |S|<=2 subsets (120 pairs + 16 singles)

C_INQ = N + 1            # 17: sentinel + 16 packed singles, per plane
C_P2Q = 1 + M2_COLS      # 137: sentinel + M2

# packing constants
MAGIC = 12582912.0       # 1.5 * 2^23: x + MAGIC - MAGIC rounds x to integer
SCALE_Q = 16384.0        # 2^14 key quantization
SCALE_V = 256.0          # payload quantization
INV_V = 1.0 / 256.0
VMAX = 255.0
C0_UNPACK = 0.498046875  # 0.5 - 2^-9: tie-free truncating round offset
SENT = 131072.0          # sentinel key: larger than any packed value


# ----------------------------------------------------------------------------
# Custom DVE op: out = min(in0,in1) - rtni(min(in0,in1) - C0 [+C1 -C1])
# ----------------------------------------------------------------------------
_OP_NAME = "MIN_UNPACK_ANT66"


def _min_unpack_ref(in0, in1, s0, s1, imm2):
    m = np.minimum(np.asarray(in0, np.float32), np.asarray(in1, np.float32))
    a = (m - np.float32(s0)).astype(np.float32)
    b = (a + np.float32(s1)).astype(np.float32)
    t = (b - np.float32(s1)).astype(np.float32)
    return (m - t).astype(np.float32)


def _register_min_unpack():
    from concourse import dve_ops
    from concourse.dve_spec import Spec, Src0, Src1, C0, C1, minn, lower
    from concourse.dve_uop import DveOpSpec

    if _OP_NAME in dve_ops._SUB_OPCODE_FOR_NAME:
        return next(o for o in dve_ops.OPS if o.name == _OP_NAME)
    m = minn(Src0, Src1)
    body = m - (((m - C0) + C1) - C1)
    spec = Spec(body=body, reference=_min_unpack_ref)
    row = max(dve_ops._SUB_OPCODE_FOR_NAME.values()) + 1
    assert row < 0x20, "no free custom-DVE opcode row"
    sha = {}
    for ver in ("v3", "v4"):
        try:
            sha[ver] = DveOpSpec(
                name=_OP_NAME, opcode=row, uops=lower(spec, ver=ver), rd1_en=True
            ).sha(ver)
        except Exception:
            pass
    op = dve_ops.DveOp(_OP_NAME, spec, subdim=False, uops_sha=sha)
    dve_ops.OPS.append(op)
    dve_ops.CUSTOM_DVE_SPECS[_OP_NAME] = spec
    dve_ops._SUB_OPCODE_FOR_NAME[_OP_NAME] = row
    return op


# ----------------------------------------------------------------------------
# Bass program builder
# ----------------------------------------------------------------------------
def build_program(rows=ROWS_PER_CORE, nb=NB_DEFAULT, reps=1):
    """Build the per-core Bass program. rows must be divisible by 128*nb.

    reps repeats the whole computation in-program (benchmarking only).
    """
    from contextlib import ExitStack
    from concourse import bacc, mybir, tile

    min_unpack = _register_min_unpack()

    f32 = mybir.dt.float32
    alu = mybir.AluOpType

    chunks = rows // (P * nb)
    assert chunks * P * nb == rows

    nc = bacc.Bacc()

    def _register_const(value):
        key = (f32, float(value))
        if key in nc.const_aps.aps:
            return
        t = nc.alloc_sbuf_tensor(f"const-f32-{value}", [128, 1], f32)
        nc.gpsimd.memset(t.ap(), float(value))
        nc.const_aps.aps[key] = t.ap()

    for _v in (MAGIC, -MAGIC, VMAX + MAGIC, VMAX):
        _register_const(_v)
    nc.all_engine_barrier()

    xl_d = nc.declare_dram_parameter("xl", [rows, N], f32, isOutput=False)
    xu_d = nc.declare_dram_parameter("xu", [rows, N], f32, isOutput=False)
    ol_d = nc.declare_dram_parameter("out_l", [rows, OUT_COLS], f32, isOutput=True)
    ou_d = nc.declare_dram_parameter("out_u", [rows, OUT_COLS], f32, isOutput=True)

    # DRAM views: (chunk, partition, rowblock, col)
    xl_r = xl_d[:].rearrange("(c nb p) t -> c p nb t", nb=nb, p=P)
    xu_r = xu_d[:].rearrange("(c nb p) t -> c p nb t", nb=nb, p=P)
    ol_r = ol_d[:].rearrange("(c nb p) o -> c p nb o", nb=nb, p=P)
    ou_r = ou_d[:].rearrange("(c nb p) o -> c p nb o", nb=nb, p=P)

    with ExitStack() as ctx:
        tc = ctx.enter_context(tile.TileContext(nc))
        rawp = ctx.enter_context(tc.tile_pool(name="rawp", bufs=2))
        singp = ctx.enter_context(tc.tile_pool(name="singp", bufs=2))
        p2p = ctx.enter_context(tc.tile_pool(name="p2p", bufs=2))
        outp = ctx.enter_context(tc.tile_pool(name="outp", bufs=1))
        outpB = ctx.enter_context(tc.tile_pool(name="outpB", bufs=1))

        for _rep in range(reps):
          for ch in range(chunks):
            # ---- load + pack singles -------------------------------------
            raw = rawp.tile([P, 2 * nb * N], f32, tag="raw")
            raw4 = raw[:].rearrange("p (v nb c) -> p v nb c", v=2, c=N)
            nc.sync.dma_start(out=raw4[:, 0], in_=xl_r[ch])
            nc.sync.dma_start(out=raw4[:, 1], in_=xu_r[ch])

            sq = rawp.tile([P, nb * N], f32, tag="sq")
            sq3 = sq[:].rearrange("p (nb c) -> p nb c", c=N)
            sq2 = rawp.tile([P, nb * N], f32, tag="sq2")
            sq23 = sq2[:].rearrange("p (nb c) -> p nb c", c=N)
            ident = mybir.ActivationFunctionType.Identity
            relu = mybir.ActivationFunctionType.Relu
            # s = l + u (DVE); Q = rtni(s * 2^14) via magic adds on ACT
            nc.vector.tensor_tensor(sq3, raw4[:, 0], raw4[:, 1], alu.add)
            nc.scalar.activation(sq23, sq3, ident, bias=MAGIC, scale=SCALE_Q)
            nc.scalar.activation(sq3, sq23, ident, bias=-MAGIC, scale=1.0)

            # v8 = min(rtni(v * 256), 255) on ACT:
            #   w = v*256 + MAGIC; r = relu(255 + MAGIC - w); v8 = 255 - r
            raw3 = raw[:].rearrange("p (g c) -> p g c", c=N)  # g = (v nb)
            vw = rawp.tile([P, 2 * nb * N], f32, tag="vw")
            vw3 = vw[:].rearrange("p (g c) -> p g c", c=N)
            nc.scalar.activation(vw3, raw3, ident, bias=MAGIC, scale=SCALE_V)
            nc.scalar.activation(raw3, vw3, relu, bias=VMAX + MAGIC, scale=-1.0)
            nc.scalar.activation(vw3, raw3, ident, bias=VMAX, scale=-1.0)

            # packed singles: sing[v, nb, 1+c] = v8/256 + Q  (q=0 sentinel)
            sing = singp.tile([P, 2 * nb * C_INQ], f32, tag="sing")
            sing4 = sing[:].rearrange("p (v nb q) -> p v nb q", v=2, q=C_INQ)
            sing3 = sing[:].rearrange("p (g q) -> p g q", q=C_INQ)
            nc.gpsimd.memset(sing4[:, 0, :, 0:1], SENT)
            nc.gpsimd.memset(sing4[:, 1, :, 0:1], SENT)
            vw4 = vw[:].rearrange("p (v nb c) -> p v nb c", v=2, c=N)
            qb = sq3.unsqueeze(1).to_broadcast((P, 2, nb, N))
            nc.vector.scalar_tensor_tensor(
                sing4[:, :, :, 1:], vw4, INV_V, qb, alu.mult, alu.add
            )

            # ---- pairs stage: M2 staging via prefix mins ------------------
            p2 = p2p.tile([P, 2 * nb * C_P2Q], f32, tag="p2")
            p24 = p2[:].rearrange("p (v nb q) -> p v nb q", v=2, q=C_P2Q)
            p23 = p2[:].rearrange("p (g q) -> p g q", q=C_P2Q)
            nc.gpsimd.memset(p24[:, 0, :, 0:1], SENT)
            nc.gpsimd.memset(p24[:, 1, :, 0:1], SENT)
            for j in range(N):
                W = j + 1
                q0 = 1 + C2[j]
                rs = sing4[:, :, :, 1 + j:2 + j].to_broadcast((P, 2, nb, W))
                nc.vector.tensor_tensor(
                    p24[:, :, :, q0:q0 + W], sing4[:, :, :, 0:W], rs, alu.min
                )

            # ---- final stage: fused min+unpack into output blocks ---------
            T_SPLIT = 14
            wA = BOFF[T_SPLIT]
            wB = OUT_COLS - wA
            osbA = outp.tile([P, 2 * nb * wA], f32, tag="osbA")
            osbB = outpB.tile([P, 2 * nb * wB], f32, tag="osbB")
            oA3 = osbA[:].rearrange("p (g c) -> p g c", c=wA)
            oB3 = osbB[:].rearrange("p (g c) -> p g c", c=wB)
            oA4 = osbA[:].rearrange("p (v nb c) -> p v nb c", v=2, c=wA)
            oB4 = osbB[:].rearrange("p (v nb c) -> p v nb c", v=2, c=wB)

            for t in range(N):
                W = C2[t] + 1
                b0 = BOFF[t]
                rs = sing3[:, :, 1 + t:2 + t].to_broadcast((P, 2 * nb, W))
                if t < T_SPLIT:
                    dst = oA3[:, :, b0:b0 + W]
                else:
                    dst = oB3[:, :, b0 - wA:b0 - wA + W]
                nc.vector._custom_dve(
                    min_unpack, out=dst, in0=p23[:, :, 0:W], in1=rs,
                    s0=C0_UNPACK, s1=MAGIC,
                )
                if t == T_SPLIT - 1:
                    # wave A out-DMAs overlap wave-B compute
                    nc.sync.dma_start(out=ol_r[ch][:, :, :wA], in_=oA4[:, 0])
                    nc.scalar.dma_start(out=ou_r[ch][:, :, :wA], in_=oA4[:, 1])

            nc.sync.dma_start(out=ol_r[ch][:, :, wA:], in_=oB4[:, 0])
            nc.scalar.dma_start(out=ou_r[ch][:, :, wA:], in_=oB4[:, 1])

    nc.finalize()
    return nc


# ----------------------------------------------------------------------------
# Exact reference semantics in numpy (for bucket-collision rows)
# ----------------------------------------------------------------------------
def _build_plan():
    from itertools import combinations

    items = list(range(N))
    index_dict = {(i,): i for i in items}
    count = N
    plan = []
    for length in range(2, min(ADD, N) + 1):
        combos = list(combinations(items, length))
        left = np.array([index_dict[c[1:]] for c in combos], dtype=np.int32)
        right = np.array([index_dict[c[:-1]] for c in combos], dtype=np.int32)
        for c in combos:
            index_dict[c] = count
            count += 1
        plan.append((left, right))

    def bitmask(c):
        m = 0
        for i in c:
            m |= 1 << i
        return m

    order = np.array(
        [index_dict[c] for c in sorted(index_dict, key=bitmask)], dtype=np.int32
    )
    return plan, order


_PLAN_CACHE = None


def _reference_numpy(xl, xu):
    """Bit-exact fp32 reproduction of the jax reference for given rows."""
    global _PLAN_CACHE
    if _PLAN_CACHE is None:
        _PLAN_CACHE = _build_plan()
    plan, order = _PLAN_CACHE
    a0 = np.float32(1.0 - ALPHA)
    a1 = np.float32(ALPHA)
    b0 = np.float32(1.0 - BETA)
    b1 = np.float32(BETA)
    mat_l, mat_u = xl.astype(np.float32), xu.astype(np.float32)
    for left_idx, right_idx in plan:
        ll, lu = mat_l[:, left_idx], mat_u[:, left_idx]
        rl, ru = mat_l[:, right_idx], mat_u[:, right_idx]
        cur = a0 * ll + a1 * lu
        nxt = a0 * rl + a1 * ru
        bcur = b0 * ll + b1 * lu
        bnxt = b0 * rl + b1 * ru
        choose_right = np.where(cur == nxt, bcur > bnxt, cur > nxt)
        res_l = np.where(choose_right, rl, ll)
        res_u = np.where(choose_right, ru, lu)
        mat_l = np.concatenate([mat_l, res_l], axis=1)
        mat_u = np.concatenate([mat_u, res_u], axis=1)
    return mat_l[:, order], mat_u[:, order]


# ----------------------------------------------------------------------------
# Host entry point
# ----------------------------------------------------------------------------
_PROGRAM_CACHE = {}


def _get_program(rows, nb):
    key = (rows, nb)
    if key not in _PROGRAM_CACHE:
        _PROGRAM_CACHE[key] = build_program(rows, nb)
    return _PROGRAM_CACHE[key]


def kernel(xl, xu):
    from concourse.bass_utils import run_bass_kernel_spmd

    xl = np.ascontiguousarray(np.asarray(xl), dtype=np.float32)
    xu = np.ascontiguousarray(np.asarray(xu), dtype=np.float32)
    assert xl.shape == (BATCH, N) and xu.shape == (BATCH, N)

    nc = _get_program(ROWS_PER_CORE, NB_DEFAULT)

    in_maps = []
    for c in range(N_CORES):
        sl = slice(c * ROWS_PER_CORE, (c + 1) * ROWS_PER_CORE)
        in_maps.append({"xl": xl[sl], "xu": xu[sl]})

    res = run_bass_kernel_spmd(nc, in_maps, list(range(N_CORES))).results

    out_l = np.concatenate([r["out_l"] for r in res], axis=0)
    out_u = np.concatenate([r["out_u"] for r in res], axis=0)

    # Patch rows where two distinct columns land in the same (or adjacent,
    # as rounding-mode insurance) key bucket: the device orders by quantized
    # key only.  ~0.5-1% of rows; exact recompute on host.
    s = (xl + xu).astype(np.float32)
    t1 = (s * np.float32(SCALE_Q)).astype(np.float32)
    q = ((t1 + np.float32(MAGIC)) - np.float32(MAGIC)).astype(np.float32)
    qs = np.sort(q, axis=1)
    bad = (np.diff(qs, axis=1) <= 1.0).any(axis=1)
    rows = np.nonzero(bad)[0]
    if rows.size:
        pl, pu = _reference_numpy(xl[rows], xu[rows])
        out_l[rows] = pl
        out_u[rows] = pu

    return out_l, out_u


# revision 7
# speedup vs baseline: 1.5001x; 1.5001x over previous
"""Trainium2 Bass kernel for nn_Min_interval (subset min-interval selection).

Problem: for each batch row, for every subset S of the 16 input columns with
|S| <= 3, output the (xl, xu) interval of the column in S minimizing the
score s = 0.5*xl + 0.5*xu (ties via b = 0.2*xl + 0.8*xu, then DP fold
order).  Output columns are sorted by subset bitmask -> [B, 696] x 2.

Packed-key min formulation (v2).  The baseline kernel materialized every
select as is_gt mask + plain copy + copy_predicated, which made the Vector
engine the bottleneck (~2650 DVE element-cycles/row, ~190 us vs the ~127 us
per-core HBM write roofline).  v2 packs key and payload into one fp32 so the
whole DP collapses to tensor_tensor MIN ops plus one fused custom-DVE op:

  *  key: Q = round(s * 2^14), s = xl+xu (compares identically to the
     0.5-weighted score).  Q in [0, 2^15], an exact fp32 integer.
  *  payload: v8 = clamp(round(v * 256), 0, 255) for v in {xl, xu}.
  *  packed: P = Q + v8/256 - exactly representable (<= 24 mantissa bits),
     so min(P) orders by (Q, v8) lexicographically and argmin Q = argmin s
     whenever all 16 row buckets are distinct.
  *  Rows where any two columns share a Q bucket (~0.5% of rows) are
     recomputed exactly on host; on all other rows the device winner equals
     the reference winner and the only error is payload quantization
     (<= 2^-9 absolute, vs the 2e-2 harness gate).
  *  Unpack is fused into the final min via a registered custom DVE op
     (single 1x pass, 8-slice ALU chain):
        m = min(in0, in1); out = m - (((m - 0.498046875) + 1.5*2^23) - 1.5*2^23)
     i.e. subtract away round-to-nearest-integer(m - (0.5 - 2^-9)) = Q of the
     winner, leaving v8/256 in [0,1).  The 0.5 - 2^-9 offset makes the
     round a truncation with no representable tie.
  *  Per chunk: pack (5 small DVE ops), 16 pair-stage TT mins building the
     136-col |S|<=2 staging array M2 (sentinel-prefixed, bitmask order), and
     16 fused min+unpack ops emitting the 696 output blocks, each a uniform
     min of a contiguous M2 prefix against one broadcast single.
  *  Outputs stream to HBM in two column waves per chunk on the SP and
     Activation HWDGE queues so stores overlap compute.

Sharding: batch 65536 rows -> 8 cores x 8192 rows (data parallel, no comms).
Measured: 63 us/core HW (vs 161 us for the select-based baseline in the same
harness); DVE-bound at ~1 output element/cycle, with the 45.6 MB/core HBM
write stream (~40 us at the measured ~1.1 TB/s/core) fully overlapped.
"""

import os
import sys
import numpy as np

for _p in ("/opt/trn_rl_repo",):
    if _p not in sys.path and os.path.isdir(_p):
        sys.path.insert(0, _p)

# ----------------------------------------------------------------------------
# Problem constants (hardcoded per contest rules)
# ----------------------------------------------------------------------------
N = 16                 # input feature columns
ADD = 3                # max subset order
ALPHA = 0.5
BETA = 0.8
BATCH = 65536
N_CORES = 8
ROWS_PER_CORE = BATCH // N_CORES        # 8192
P = 128                                 # SBUF partitions
OUT_COLS = 696                          # subsets with 1<=|S|<=3 of 16
NB_DEFAULT = 16                         # rowblocks per chunk

# triangular prefix counts: c2[t] = #subsets |S|<=2 with bitmask < 2^t
C2 = [t * (t + 1) // 2 for t in range(N + 1)]
# output block offsets: block t (top bit t) starts at BOFF[t], width 1+c2[t]
BOFF = [0] * (N + 1)
for _t in range(N):
    BOFF[_t + 1] = BOFF[_t] + 1 + C2[_t]
assert BOFF[N] == OUT_COLS

M2_COLS = C2[N]  # 136 = number of |S|<=2 subsets (120 pairs + 16 singles)

C_INQ = N + 1            # 17: sentinel + 16 packed singles, per plane
C_P2Q = 1 + M2_COLS      # 137: sentinel + M2

# packing constants
MAGIC = 12582912.0       # 1.5 * 2^23: x + MAGIC - MAGIC rounds x to integer
SCALE_Q = 16384.0        # 2^14 key quantization
SCALE_V = 256.0          # payload quantization
INV_V = 1.0 / 256.0
VMAX = 255.0
C0_UNPACK = 0.498046875  # 0.5 - 2^-9: tie-free truncating round offset
SENT = 131072.0          # sentinel key: larger than any packed value


# ----------------------------------------------------------------------------
# Custom DVE op: out = min(in0,in1) - rtni(min(in0,in1) - C0 [+C1 -C1])
# ----------------------------------------------------------------------------
_OP_NAME = "MIN_UNPACK_ANT66"


def _min_unpack_ref(in0, in1, s0, s1, imm2):
    m = np.minimum(np.asarray(in0, np.float32), np.asarray(in1, np.float32))
    a = (m - np.float32(s0)).astype(np.float32)
    b = (a + np.float32(s1)).astype(np.float32)
    t = (b - np.float32(s1)).astype(np.float32)
    return (m - t).astype(np.float32)


def _register_min_unpack():
    from concourse import dve_ops
    from concourse.dve_spec import Spec, Src0, Src1, C0, C1, minn, lower
    from concourse.dve_uop import DveOpSpec

    if _OP_NAME in dve_ops._SUB_OPCODE_FOR_NAME:
        return next(o for o in dve_ops.OPS if o.name == _OP_NAME)
    m = minn(Src0, Src1)
    body = m - (((m - C0) + C1) - C1)
    spec = Spec(body=body, reference=_min_unpack_ref)
    row = max(dve_ops._SUB_OPCODE_FOR_NAME.values()) + 1
    assert row < 0x20, "no free custom-DVE opcode row"
    sha = {}
    for ver in ("v3", "v4"):
        try:
            sha[ver] = DveOpSpec(
                name=_OP_NAME, opcode=row, uops=lower(spec, ver=ver), rd1_en=True
            ).sha(ver)
        except Exception:
            pass
    op = dve_ops.DveOp(_OP_NAME, spec, subdim=False, uops_sha=sha)
    dve_ops.OPS.append(op)
    dve_ops.CUSTOM_DVE_SPECS[_OP_NAME] = spec
    dve_ops._SUB_OPCODE_FOR_NAME[_OP_NAME] = row
    return op


# ----------------------------------------------------------------------------
# Bass program builder
# ----------------------------------------------------------------------------
def build_program(rows=ROWS_PER_CORE, nb=NB_DEFAULT, reps=1):
    """Build the per-core Bass program. rows must be divisible by 128*nb.

    reps repeats the whole computation in-program (benchmarking only).
    """
    from contextlib import ExitStack
    from concourse import bacc, mybir, tile

    min_unpack = _register_min_unpack()

    f32 = mybir.dt.float32
    alu = mybir.AluOpType

    chunks = rows // (P * nb)
    assert chunks * P * nb == rows

    nc = bacc.Bacc()
    xl_d = nc.declare_dram_parameter("xl", [rows, N], f32, isOutput=False)
    xu_d = nc.declare_dram_parameter("xu", [rows, N], f32, isOutput=False)
    ol_d = nc.declare_dram_parameter("out_l", [rows, OUT_COLS], f32, isOutput=True)
    ou_d = nc.declare_dram_parameter("out_u", [rows, OUT_COLS], f32, isOutput=True)

    # DRAM views: (chunk, partition, rowblock, col)
    xl_r = xl_d[:].rearrange("(c nb p) t -> c p nb t", nb=nb, p=P)
    xu_r = xu_d[:].rearrange("(c nb p) t -> c p nb t", nb=nb, p=P)
    ol_r = ol_d[:].rearrange("(c nb p) o -> c p nb o", nb=nb, p=P)
    ou_r = ou_d[:].rearrange("(c nb p) o -> c p nb o", nb=nb, p=P)

    with ExitStack() as ctx:
        tc = ctx.enter_context(tile.TileContext(nc))
        rawp = ctx.enter_context(tc.tile_pool(name="rawp", bufs=2))
        singp = ctx.enter_context(tc.tile_pool(name="singp", bufs=2))
        p2p = ctx.enter_context(tc.tile_pool(name="p2p", bufs=2))
        outp = ctx.enter_context(tc.tile_pool(name="outp", bufs=1))
        outpB = ctx.enter_context(tc.tile_pool(name="outpB", bufs=1))

        for _rep in range(reps):
          for ch in range(chunks):
            # ---- load + pack singles -------------------------------------
            raw = rawp.tile([P, 2 * nb * N], f32, tag="raw")
            raw4 = raw[:].rearrange("p (v nb c) -> p v nb c", v=2, c=N)
            nc.sync.dma_start(out=raw4[:, 0], in_=xl_r[ch])
            nc.sync.dma_start(out=raw4[:, 1], in_=xu_r[ch])

            sq = rawp.tile([P, nb * N], f32, tag="sq")
            sq3 = sq[:].rearrange("p (nb c) -> p nb c", c=N)
            # s = l + u; then Q = rtni(s * 2^14) via the magic-constant trick
            nc.vector.tensor_tensor(sq3, raw4[:, 0], raw4[:, 1], alu.add)
            nc.vector.tensor_scalar(sq3, sq3, SCALE_Q, MAGIC, alu.mult, alu.add)
            nc.vector.tensor_scalar(sq3, sq3, MAGIC, None, alu.subtract)

            # v8 = min(rtni(v * 256), 255), in place in raw (both planes)
            raw3 = raw[:].rearrange("p (g c) -> p g c", c=N)  # g = (v nb)
            nc.vector.tensor_scalar(raw3, raw3, SCALE_V, MAGIC, alu.mult, alu.add)
            nc.vector.tensor_scalar(raw3, raw3, MAGIC, VMAX, alu.subtract, alu.min)

            # packed singles: sing[v, nb, 1+c] = v8/256 + Q  (q=0 sentinel)
            sing = singp.tile([P, 2 * nb * C_INQ], f32, tag="sing")
            sing4 = sing[:].rearrange("p (v nb q) -> p v nb q", v=2, q=C_INQ)
            sing3 = sing[:].rearrange("p (g q) -> p g q", q=C_INQ)
            nc.gpsimd.memset(sing4[:, 0, :, 0:1], SENT)
            nc.gpsimd.memset(sing4[:, 1, :, 0:1], SENT)
            qb = sq3.unsqueeze(1).to_broadcast((P, 2, nb, N))
            nc.vector.scalar_tensor_tensor(
                sing4[:, :, :, 1:], raw4, INV_V, qb, alu.mult, alu.add
            )

            # ---- pairs stage: M2 staging via prefix mins ------------------
            p2 = p2p.tile([P, 2 * nb * C_P2Q], f32, tag="p2")
            p24 = p2[:].rearrange("p (v nb q) -> p v nb q", v=2, q=C_P2Q)
            p23 = p2[:].rearrange("p (g q) -> p g q", q=C_P2Q)
            nc.gpsimd.memset(p24[:, 0, :, 0:1], SENT)
            nc.gpsimd.memset(p24[:, 1, :, 0:1], SENT)
            for j in range(N):
                W = j + 1
                q0 = 1 + C2[j]
                rs = sing4[:, :, :, 1 + j:2 + j].to_broadcast((P, 2, nb, W))
                nc.vector.tensor_tensor(
                    p24[:, :, :, q0:q0 + W], sing4[:, :, :, 0:W], rs, alu.min
                )

            # ---- final stage: fused min+unpack into output blocks ---------
            T_SPLIT = 14
            wA = BOFF[T_SPLIT]
            wB = OUT_COLS - wA
            osbA = outp.tile([P, 2 * nb * wA], f32, tag="osbA")
            osbB = outpB.tile([P, 2 * nb * wB], f32, tag="osbB")
            oA3 = osbA[:].rearrange("p (g c) -> p g c", c=wA)
            oB3 = osbB[:].rearrange("p (g c) -> p g c", c=wB)
            oA4 = osbA[:].rearrange("p (v nb c) -> p v nb c", v=2, c=wA)
            oB4 = osbB[:].rearrange("p (v nb c) -> p v nb c", v=2, c=wB)

            for t in range(N):
                W = C2[t] + 1
                b0 = BOFF[t]
                rs = sing3[:, :, 1 + t:2 + t].to_broadcast((P, 2 * nb, W))
                if t < T_SPLIT:
                    dst = oA3[:, :, b0:b0 + W]
                else:
                    dst = oB3[:, :, b0 - wA:b0 - wA + W]
                nc.vector._custom_dve(
                    min_unpack, out=dst, in0=p23[:, :, 0:W], in1=rs,
                    s0=C0_UNPACK, s1=MAGIC,
                )
                if t == T_SPLIT - 1:
                    # wave A out-DMAs overlap wave-B compute
                    nc.sync.dma_start(out=ol_r[ch][:, :, :wA], in_=oA4[:, 0])
                    nc.scalar.dma_start(out=ou_r[ch][:, :, :wA], in_=oA4[:, 1])

            nc.sync.dma_start(out=ol_r[ch][:, :, wA:], in_=oB4[:, 0])
            nc.scalar.dma_start(out=ou_r[ch][:, :, wA:], in_=oB4[:, 1])

    nc.finalize()
    return nc


# ----------------------------------------------------------------------------
# Exact reference semantics in numpy (for bucket-collision rows)
# ----------------------------------------------------------------------------
def _build_plan():
    from itertools import combinations

    items = list(range(N))
    index_dict = {(i,): i for i in items}
    count = N
    plan = []
    for length in range(2, min(ADD, N) + 1):
        combos = list(combinations(items, length))
        left = np.array([index_dict[c[1:]] for c in combos], dtype=np.int32)
        right = np.array([index_dict[c[:-1]] for c in combos], dtype=np.int32)
        for c in combos:
            index_dict[c] = count
            count += 1
        plan.append((left, right))

    def bitmask(c):
        m = 0
        for i in c:
            m |= 1 << i
        return m

    order = np.array(
        [index_dict[c] for c in sorted(index_dict, key=bitmask)], dtype=np.int32
    )
    return plan, order


_PLAN_CACHE = None


def _reference_numpy(xl, xu):
    """Bit-exact fp32 reproduction of the jax reference for given rows."""
    global _PLAN_CACHE
    if _PLAN_CACHE is None:
        _PLAN_CACHE = _build_plan()
    plan, order = _PLAN_CACHE
    a0 = np.float32(1.0 - ALPHA)
    a1 = np.float32(ALPHA)
    b0 = np.float32(1.0 - BETA)
    b1 = np.float32(BETA)
    mat_l, mat_u = xl.astype(np.float32), xu.astype(np.float32)
    for left_idx, right_idx in plan:
        ll, lu = mat_l[:, left_idx], mat_u[:, left_idx]
        rl, ru = mat_l[:, right_idx], mat_u[:, right_idx]
        cur = a0 * ll + a1 * lu
        nxt = a0 * rl + a1 * ru
        bcur = b0 * ll + b1 * lu
        bnxt = b0 * rl + b1 * ru
        choose_right = np.where(cur == nxt, bcur > bnxt, cur > nxt)
        res_l = np.where(choose_right, rl, ll)
        res_u = np.where(choose_right, ru, lu)
        mat_l = np.concatenate([mat_l, res_l], axis=1)
        mat_u = np.concatenate([mat_u, res_u], axis=1)
    return mat_l[:, order], mat_u[:, order]


# ----------------------------------------------------------------------------
# Host entry point
# ----------------------------------------------------------------------------
_PROGRAM_CACHE = {}


def _get_program(rows, nb):
    key = (rows, nb)
    if key not in _PROGRAM_CACHE:
        _PROGRAM_CACHE[key] = build_program(rows, nb)
    return _PROGRAM_CACHE[key]


def kernel(xl, xu):
    from concourse.bass_utils import run_bass_kernel_spmd

    xl = np.ascontiguousarray(np.asarray(xl), dtype=np.float32)
    xu = np.ascontiguousarray(np.asarray(xu), dtype=np.float32)
    assert xl.shape == (BATCH, N) and xu.shape == (BATCH, N)

    nc = _get_program(ROWS_PER_CORE, NB_DEFAULT)

    in_maps = []
    for c in range(N_CORES):
        sl = slice(c * ROWS_PER_CORE, (c + 1) * ROWS_PER_CORE)
        in_maps.append({"xl": xl[sl], "xu": xu[sl]})

    res = run_bass_kernel_spmd(nc, in_maps, list(range(N_CORES))).results

    out_l = np.concatenate([r["out_l"] for r in res], axis=0)
    out_u = np.concatenate([r["out_u"] for r in res], axis=0)

    # Patch rows where two distinct columns land in the same (or adjacent,
    # as rounding-mode insurance) key bucket: the device orders by quantized
    # key only.  ~0.5-1% of rows; exact recompute on host.
    s = (xl + xu).astype(np.float32)
    t1 = (s * np.float32(SCALE_Q)).astype(np.float32)
    q = ((t1 + np.float32(MAGIC)) - np.float32(MAGIC)).astype(np.float32)
    qs = np.sort(q, axis=1)
    bad = (np.diff(qs, axis=1) <= 1.0).any(axis=1)
    rows = np.nonzero(bad)[0]
    if rows.size:
        pl, pu = _reference_numpy(xl[rows], xu[rows])
        out_l[rows] = pl
        out_u[rows] = pu

    return out_l, out_u
